# revision 2
# baseline (speedup 1.0000x reference)
"""Trainium2 kernel for nn_COVIDetector (sparse LSH attention net).

v0: data-parallel execution of the model via jax/PJRT across the axon
NeuronCores (batch sharding). Serves as correctness baseline while the
Bass kernel is developed.
"""

import numpy as np
import jax
import jax.numpy as jnp
from jax import lax

HEADS = 8
N_HASHES = 4
BUCKET = 64
DEPTH = 2
DIM = 128
EPS = 1e-5
SELF_ATTN_VAL = -5e4


def _layer_norm(x, g, b):
    mu = jnp.mean(x, -1, keepdims=True)
    var = jnp.mean((x - mu) ** 2, -1, keepdims=True)
    return (x - mu) * lax.rsqrt(var + EPS) * g + b


def _lsh_attention(x, wqk, wv, wo, wo_b, rot):
    B, T, D = x.shape
    dh = D // HEADS
    nb = T // BUCKET

    def split_heads(h):
        return h.reshape(B, T, HEADS, dh).transpose(0, 2, 1, 3).reshape(B * HEADS, T, dh)

    qk = split_heads(x @ wqk)
    v = split_heads(x @ wv)

    rotated = jnp.einsum('btf,fhr->bhtr', qk, rot)
    rotated = jnp.concatenate([rotated, -rotated], axis=-1)
    buckets = jnp.argmax(rotated, axis=-1)
    buckets = buckets + (jnp.arange(N_HASHES) * nb)[None, :, None]
    buckets = buckets.reshape(B * HEADS, N_HASHES * T)

    # --- arithmetic stable counting sort (trn2 has no HLO sort) ---
    # dest[i] = #{j: bucket[j] < bucket[i]} + #{j < i: bucket[j] == bucket[i]}
    nbk = N_HASHES * nb  # 32 global buckets
    oh = (buckets[..., None] == jnp.arange(nbk)[None, None, :]).astype(jnp.float32)
    cum_incl = jnp.cumsum(oh, axis=1)                  # (BH, 4T, 32)
    counts = cum_incl[:, -1, :]                        # (BH, 32)
    offs = jnp.cumsum(counts, axis=-1) - counts        # exclusive over buckets
    rank = jnp.sum(oh * cum_incl, axis=-1) - 1.0       # (BH, 4T)
    dest = (jnp.sum(oh * offs[:, None, :], axis=-1) + rank).astype(jnp.int32)
    undo = dest

    # forward sort = scatter rows by dest
    qk_rep = jnp.tile(qk, (1, N_HASHES, 1))
    v_rep = jnp.tile(v, (1, N_HASHES, 1))
    t_rep = jnp.tile(jnp.arange(T), N_HASHES)[None, :].repeat(B * HEADS, 0)
    sqk = jnp.zeros_like(qk_rep).at[
        jnp.arange(B * HEADS)[:, None], dest, :].set(qk_rep)
    sv = jnp.zeros_like(v_rep).at[
        jnp.arange(B * HEADS)[:, None], dest, :].set(v_rep)
    st = jnp.zeros_like(t_rep).at[
        jnp.arange(B * HEADS)[:, None], dest].set(t_rep)

    nchunks = N_HASHES * nb
    bq_t = st.reshape(B * HEADS, nchunks, BUCKET)
    bqk = sqk.reshape(B * HEADS, nchunks, BUCKET, dh)
    bv = sv.reshape(B * HEADS, nchunks, BUCKET, dh)
    bq = bqk
    bk = bqk / jnp.maximum(jnp.linalg.norm(bqk, axis=-1, keepdims=True), 1e-12)

    def look_one_back(a):
        return jnp.concatenate([a, jnp.roll(a, 1, axis=1)], axis=2)

    bk = look_one_back(bk)
    bv = look_one_back(bv)
    bkv_t = look_one_back(bq_t)

    dots = jnp.einsum('bcie,bcje->bcij', bq, bk) * (dh ** -0.5)
    dots = jnp.where(bq_t[..., :, None] == bkv_t[..., None, :], SELF_ATTN_VAL, dots)

    lse = jax.scipy.special.logsumexp(dots, axis=-1, keepdims=True)
    bo = jnp.einsum('bcij,bcje->bcie', jnp.exp(dots - lse), bv)

    so = bo.reshape(B * HEADS, N_HASHES * T, dh)
    slog = lse.reshape(B * HEADS, N_HASHES * T)
    o = jnp.take_along_axis(so, undo[..., None], axis=1).reshape(B * HEADS, N_HASHES, T, dh)
    logits = jnp.take_along_axis(slog, undo, axis=1).reshape(B * HEADS, N_HASHES, T, 1)
    probs = jnp.exp(logits - jax.scipy.special.logsumexp(logits, axis=1, keepdims=True))
    out = jnp.sum(o * probs, axis=1)
    out = out.reshape(B, HEADS, T, dh).transpose(0, 2, 1, 3).reshape(B, T, D)
    return out @ wo + wo_b


def _model(x, conv1_w, conv1_b, conv2_w, conv2_b, conv3_w, conv3_b,
           conv4_w, conv4_b, conv5_w, conv5_b,
           ln1_g, ln1_b, ln2_g, ln2_b, wqk, wv, wo, wo_b,
           ff_w1, ff_b1, ff_w2, ff_b2, rotations, lin_w, lin_b):
    dn = ('NCHW', 'OIHW', 'NCHW')

    def conv(t, w, b):
        return lax.conv_general_dilated(t, w, (2, 2), 'VALID', dimension_numbers=dn) + b[None, :, None, None]

    def pool(t):
        return lax.reduce_window(t, -jnp.inf, lax.max, (1, 1, 2, 2), (1, 1, 2, 2), 'VALID')

    gelu = lambda t: jax.nn.gelu(t, approximate=False)

    t = pool(gelu(conv(x, conv1_w, conv1_b)))
    t = pool(gelu(conv(t, conv2_w, conv2_b)))
    t = pool(gelu(conv(t, conv3_w, conv3_b)))
    B, C, S, K = t.shape
    seq = t.transpose(0, 2, 3, 1).reshape(B, S * K, C)

    x1 = x2 = seq
    for d in range(DEPTH):
        y1 = x1 + _lsh_attention(_layer_norm(x2, ln1_g[d], ln1_b[d]),
                                 wqk[d], wv[d], wo[d], wo_b[d], rotations[d])
        h = _layer_norm(y1, ln2_g[d], ln2_b[d])
        y2 = x2 + (gelu(h @ ff_w1[d] + ff_b1[d]) @ ff_w2[d] + ff_b2[d])
        x1, x2 = y1, y2
    seq = (x1 + x2) * 0.5

    t = seq.reshape(B, S, K, C).transpose(0, 3, 1, 2)
    t = pool(gelu(conv(t, conv4_w, conv4_b)))
    t = pool(gelu(conv(t, conv5_w, conv5_b)))
    return t.reshape(B, -1) @ lin_w + lin_b


_ARG_ORDER = [
    "x", "conv1_w", "conv1_b", "conv2_w", "conv2_b", "conv3_w", "conv3_b",
    "conv4_w", "conv4_b", "conv5_w", "conv5_b",
    "ln1_g", "ln1_b", "ln2_g", "ln2_b", "wqk", "wv", "wo", "wo_b",
    "ff_w1", "ff_b1", "ff_w2", "ff_b2", "rotations", "lin_w", "lin_b",
]

_jitted = None


def kernel(**inputs: np.ndarray) -> np.ndarray:
    global _jitted
    devs = jax.devices()[:4]
    args = [np.asarray(inputs[k]) for k in _ARG_ORDER]
    if _jitted is None:
        _jitted = jax.pmap(_model, axis_name="b", devices=devs,
                           in_axes=tuple([0] + [None] * (len(_ARG_ORDER) - 1)))
    x = args[0]
    xs = x.reshape(4, 1, *x.shape[1:])
    out = _jitted(xs, *args[1:])
    return np.asarray(out).reshape(4, -1).astype(np.float32)
